# revision 3
# baseline (speedup 1.0000x reference)
"""DeeperGCN forward on 8 TRN2 NeuronCores.

Math (exact algebraic collapse of the reference):
  m_e   = relu(feat[src]) (+eps dropped: |effect| < 1e-6 relative)
  per (dst node n, dim d):  softmax-weighted mean over incoming edges
     denom[n,d] = sum_e exp(m_e),   num[n,d] = sum_e m_e*exp(m_e)
     agg[n,d]   = num/denom         (0 for isolated nodes)
  hv_final = feat + (feat+agg) @ (W0+W1+W2) + sum(bl)    [layers read original feat]
  out = [mean(hv_final,0)] @ Wout + bout
      = f(mean(feat,0), S) where S[d] = sum_n agg[n,d]

Device computes: S_partial and sum(feat)_partial per core; host finishes the
tiny [128]-vector epilogue.

Sharding: dst-node windows of 128 nodes (400 windows over padded 51200-node
range), load-balanced 50 windows per core. Per window: indirect-DMA gather of
edge source rows, exp/q on ACT/DVE, per-128-edge-tile one-hot (iota==dst_local)
matmul accumulating [denom|num] in PSUM, then ratio+accumulate epilogue.
"""
import math

import numpy as np

P = 128
N_NODES = 50000
N_EDGES = 800000
D = 128
N_CORES = 8
NWIN_TOTAL = 400          # 400 * 128 = 51200 >= 50000
NWIN_CORE = NWIN_TOTAL // N_CORES   # 50
NODES_CORE = NWIN_CORE * P          # 6400

_CACHE = {}


def _build(W_T):
    import concourse.bacc as bacc
    import concourse.tile as tile
    from concourse import bass, mybir
    from concourse.bass import IndirectOffsetOnAxis

    f32 = mybir.dt.float32
    i32 = mybir.dt.int32
    WE = W_T * P
    T = NWIN_CORE * W_T

    nc = bacc.Bacc("TRN2", target_bir_lowering=False, debug=False,
                   num_devices=N_CORES)
    feat_d = nc.dram_tensor("feat", [N_NODES, D], f32, kind="ExternalInput")
    srcw_d = nc.dram_tensor("srcw", [P, T], i32, kind="ExternalInput")
    dstl_d = nc.dram_tensor("dstl", [P, T], f32, kind="ExternalInput")
    fsl_d = nc.dram_tensor("fsl", [NODES_CORE, D], f32, kind="ExternalInput")
    outp_d = nc.dram_tensor("outp", [P, 2], f32, kind="ExternalOutput")

    with tile.TileContext(nc) as tc:
        with tc.tile_pool(name="const", bufs=1) as cst, \
             tc.tile_pool(name="g", bufs=3) as gp, \
             tc.tile_pool(name="rhs", bufs=3) as rp, \
             tc.tile_pool(name="oh", bufs=3) as ohp, \
             tc.tile_pool(name="ep", bufs=2) as epp, \
             tc.tile_pool(name="fs", bufs=3) as fsp, \
             tc.tile_pool(name="ps", bufs=4, space="PSUM") as psp, \
             tc.tile_pool(name="psr", bufs=1, space="PSUM") as psrp:

            srcw = cst.tile([P, T], i32)
            nc.sync.dma_start(srcw[:], srcw_d.ap())
            dstl = cst.tile([P, T], f32)
            nc.sync.dma_start(dstl[:], dstl_d.ap())
            iota = cst.tile([P, P], f32)
            nc.gpsimd.iota(iota[:], pattern=[[1, P]], base=0,
                           channel_multiplier=0,
                           allow_small_or_imprecise_dtypes=True)
            ones = cst.tile([P, 1], f32)
            nc.vector.memset(ones[:], 1.0)
            acc = cst.tile([P, D], f32)
            nc.vector.memset(acc[:], 0.0)

            fred = psrp.tile([P, 2], f32)

            for w in range(NWIN_CORE):
                g = gp.tile([P, WE], f32)
                for t in range(W_T):
                    gt = w * W_T + t
                    nc.gpsimd.indirect_dma_start(
                        out=g[:, t * P:(t + 1) * P], out_offset=None,
                        in_=feat_d.ap(),
                        in_offset=IndirectOffsetOnAxis(
                            ap=srcw[:, gt:gt + 1], axis=0))
                # g <- relu(g);  p = exp(g); q = g*p
                nc.scalar.activation(g[:], g[:],
                                     mybir.ActivationFunctionType.Relu)
                rhs = rp.tile([P, 2 * WE], f32)
                nc.scalar.activation(rhs[:, 0:WE], g[:],
                                     mybir.ActivationFunctionType.Exp)
                nc.vector.tensor_tensor(out=rhs[:, WE:2 * WE], in0=g[:],
                                        in1=rhs[:, 0:WE],
                                        op=mybir.AluOpType.mult)
                ohw = ohp.tile([P, WE], f32)
                ps = psp.tile([P, 2 * P], f32)
                pqv = rhs[:].rearrange("p (h c) -> p h c", h=2)
                for t in range(W_T):
                    gt = w * W_T + t
                    nc.vector.tensor_scalar(
                        out=ohw[:, t * P:(t + 1) * P], in0=iota[:],
                        scalar1=dstl[:, gt:gt + 1], scalar2=None,
                        op0=mybir.AluOpType.is_equal)
                    nc.tensor.matmul(
                        ps[:], lhsT=ohw[:, t * P:(t + 1) * P],
                        rhs=pqv[:, :, t * P:(t + 1) * P],
                        start=(t == 0), stop=(t == W_T - 1))
                den = epp.tile([P, P], f32)
                nc.vector.tensor_scalar(out=den[:], in0=ps[:, 0:P],
                                        scalar1=1e-20, scalar2=None,
                                        op0=mybir.AluOpType.add)
                rec = epp.tile([P, P], f32)
                nc.vector.reciprocal(rec[:], den[:])
                ratio = epp.tile([P, P], f32)
                nc.vector.tensor_tensor(out=ratio[:], in0=ps[:, P:2 * P],
                                        in1=rec[:], op=mybir.AluOpType.mult)
                nc.vector.tensor_tensor(out=acc[:], in0=acc[:], in1=ratio[:],
                                        op=mybir.AluOpType.add)

            # feat-slice sum via PE: fred[:,1] accumulates ones-reduce
            for i in range(NWIN_CORE):
                ft = fsp.tile([P, D], f32)
                nc.sync.dma_start(ft[:], fsl_d.ap()[i * P:(i + 1) * P, :])
                nc.tensor.matmul(fred[:, 1:2], lhsT=ft[:], rhs=ones[:],
                                 start=(i == 0), stop=(i == NWIN_CORE - 1))
            # S partial = sum over node-slot partitions of acc
            nc.tensor.matmul(fred[:, 0:1], lhsT=acc[:], rhs=ones[:],
                             start=True, stop=True)
            outsb = epp.tile([P, 2], f32)
            nc.scalar.copy(outsb[:], fred[:])
            nc.sync.dma_start(outp_d.ap(), outsb[:])

    nc.compile()
    return nc


def _build_noop(W_T):
    """Same I/O signature as _build but near-empty body — for timing diff."""
    import concourse.bacc as bacc
    import concourse.tile as tile
    from concourse import mybir

    f32 = mybir.dt.float32
    i32 = mybir.dt.int32
    T = NWIN_CORE * W_T
    nc = bacc.Bacc("TRN2", target_bir_lowering=False, debug=False,
                   num_devices=N_CORES)
    nc.dram_tensor("feat", [N_NODES, D], f32, kind="ExternalInput")
    srcw_d = nc.dram_tensor("srcw", [P, T], i32, kind="ExternalInput")
    nc.dram_tensor("dstl", [P, T], f32, kind="ExternalInput")
    nc.dram_tensor("fsl", [NODES_CORE, D], f32, kind="ExternalInput")
    outp_d = nc.dram_tensor("outp", [P, 2], f32, kind="ExternalOutput")
    with tile.TileContext(nc) as tc:
        with tc.tile_pool(name="sb", bufs=1) as sb:
            t1 = sb.tile([P, 2], f32)
            nc.vector.memset(t1[:], 0.0)
            nc.sync.dma_start(outp_d.ap(), t1[:])
    nc.compile()
    return nc


def _preprocess(feat, src, dst):
    """Bucket edges by 128-node dst window; build per-core padded arrays."""
    src = np.ascontiguousarray(src, dtype=np.int64)
    dst = np.ascontiguousarray(dst, dtype=np.int64)
    win = (dst >> 7).astype(np.int64)          # dst // 128
    loc = (dst & 127).astype(np.float32)       # dst % 128
    counts = np.bincount(win, minlength=NWIN_TOTAL)
    W_T = max(1, int(math.ceil(counts.max() / P)))
    WE = W_T * P
    T = NWIN_CORE * W_T

    order = np.argsort(win, kind="stable")
    src_s = src[order].astype(np.int32)
    loc_s = loc[order]
    starts = np.zeros(NWIN_TOTAL + 1, np.int64)
    np.cumsum(counts, out=starts[1:])

    # load-balanced assignment: biggest windows first to least-loaded core
    w_order = np.argsort(-counts, kind="stable")
    core_wins = [[] for _ in range(N_CORES)]
    load = np.zeros(N_CORES, np.int64)
    for w in w_order:
        cands = [c for c in range(N_CORES) if len(core_wins[c]) < NWIN_CORE]
        c = min(cands, key=lambda c: load[c])
        core_wins[c].append(int(w))
        load[c] += counts[w]

    srcw = np.zeros((N_CORES, NWIN_CORE, WE), np.int32)
    dstl = np.full((N_CORES, NWIN_CORE, WE), -1.0, np.float32)
    fsl = np.zeros((N_CORES, NODES_CORE, D), np.float32)
    for c in range(N_CORES):
        for j, w in enumerate(core_wins[c]):
            s, e = starts[w], starts[w + 1]
            n = e - s
            srcw[c, j, :n] = src_s[s:e]
            dstl[c, j, :n] = loc_s[s:e]
            base = w * P
            hi = min(base + P, N_NODES)
            if hi > base:
                fsl[c, j * P: j * P + (hi - base)] = feat[base:hi]

    # [50, WE] -> [T, 128] tiles -> SBUF layout [128, T]
    srcw_t = np.ascontiguousarray(
        srcw.reshape(N_CORES, T, P).transpose(0, 2, 1))
    dstl_t = np.ascontiguousarray(
        dstl.reshape(N_CORES, T, P).transpose(0, 2, 1))
    return W_T, srcw_t, dstl_t, fsl


def kernel(feat, src, dst, Wl, bl, Wout, bout):
    from concourse.bass_utils import run_bass_kernel_spmd

    feat = np.ascontiguousarray(feat, dtype=np.float32)
    W_T, srcw_t, dstl_t, fsl = _preprocess(feat, src, dst)

    if W_T not in _CACHE:
        _CACHE[W_T] = _build(W_T)
    nc = _CACHE[W_T]

    in_maps = [
        {"feat": feat, "srcw": srcw_t[c], "dstl": dstl_t[c], "fsl": fsl[c]}
        for c in range(N_CORES)
    ]
    res = run_bass_kernel_spmd(nc, in_maps, core_ids=list(range(N_CORES)))

    S = np.zeros(D, np.float64)
    fsum = np.zeros(D, np.float64)
    for c in range(N_CORES):
        out = res.results[c]["outp"]
        S += out[:, 0].astype(np.float64)
        fsum += out[:, 1].astype(np.float64)

    mean_feat = (fsum / N_NODES).astype(np.float32)
    mean_agg = (S / N_NODES).astype(np.float32)
    mean_base = mean_feat + mean_agg
    Wsum = np.asarray(Wl, np.float32).sum(axis=0)
    bsum = np.asarray(bl, np.float32).sum(axis=0)
    h = mean_feat + mean_base @ Wsum + bsum
    out = h @ np.asarray(Wout, np.float32) + np.asarray(bout, np.float32)
    return out[None, :].astype(np.float32)
